# revision 10
# baseline (speedup 1.0000x reference)
"""Trainium2 Bass kernel for nn_LoretzFusion (retrieval_knn).

Strategy (8 NeuronCores, SPMD via run_bass_kernel_spmd):
  - Encoder (5 parallel MLPs + BatchNorm over N) is replicated on every
    core (cheap relative to the N x N similarity work, avoids collectives).
  - The N x N similarity work is sharded by stru rows: core k owns rows
    [k*512, (k+1)*512). Each core computes its [512, 4096] exp-sim block,
    row-sums, argmax, gathers mmn[argmax] (indirect DMA), computes its
    [512, 4096] mm-rows block, top-6 via max8/match_replace mask, and the
    per-row loss terms.
  - Host slices/transposes inputs, and merges per-core outputs (concat of
    row slices + scalar mean for the loss).
All matmuls use float32r (full-rate fp32 mode, ~1e-3 rel precision).
"""

import sys
import os

sys.path.insert(0, "/opt/trn_rl_repo")

import numpy as np
import concourse.bacc as bacc
import concourse.bass as bass
import concourse.tile as tile
from concourse import mybir
from concourse.bass_utils import run_bass_kernel_spmd

F32 = mybir.dt.float32
F32R = mybir.dt.float32r
I32 = mybir.dt.int32
U32 = mybir.dt.uint32
AF = mybir.ActivationFunctionType
OP = mybir.AluOpType

NCORES = 8
N, R, MM, C = 4096, 256, 768, 5
S = N // NCORES          # 512 stru rows per core
NB = N // 512            # 8 column blocks of 512
FC = R // 128            # 2 chunks of the 256-dim feature axis
KC = MM // 128           # 6 chunks of the 768-dim contraction
IC = S // 128            # 4 chunks of this core's 512 rows
TAU = 0.5
BN_EPS = 1e-5
NORM_EPS = 1e-12
BIGNEG = -3.0e38
OOB = 1 << 30


def build_program():
    nc = bacc.Bacc("TRN2", target_bir_lowering=False, debug=True)

    mmT_d = nc.dram_tensor("mmT", [MM, N], F32R, kind="ExternalInput")
    w1t_d = nc.dram_tensor("w1t", [C, MM, R], F32R, kind="ExternalInput")
    w2t_d = nc.dram_tensor("w2t", [C, R, R], F32R, kind="ExternalInput")
    b1_d = nc.dram_tensor("b1", [128, FC, C], F32, kind="ExternalInput")
    gamma_d = nc.dram_tensor("gamma", [128, FC, C], F32, kind="ExternalInput")
    beta_d = nc.dram_tensor("beta", [128, FC, C], F32, kind="ExternalInput")
    stT_d = nc.dram_tensor("stT", [C, R, S], F32R, kind="ExternalInput")
    rows_d = nc.dram_tensor("rows", [128, 32], I32, kind="ExternalInput")

    # N + 128 guard rows: foreign blocks scatter into the guard region
    mmout_d = [
        nc.dram_tensor(f"mm_out_{c}", [N + 128, R], F32, kind="ExternalOutput")
        for c in range(C)
    ]
    loss_d = nc.dram_tensor("loss", [128, C * IC], F32, kind="ExternalOutput")

    mmnT_dram = nc.dram_tensor("mmnT_i", [C, 128, FC, N], F32R)
    mmn_dram = [nc.dram_tensor(f"mmn_i_{c}", [N, R], F32R) for c in range(C)]
    rn_dram = nc.dram_tensor("rn_i", [128, 4], F32)

    with tile.TileContext(nc) as tc:
        with tc.tile_pool(name="const", bufs=1) as constp:
            # identity matrices for PE transposes
            io_t = constp.tile([128, 128], I32)
            nc.gpsimd.iota(io_t[:], pattern=[[1, 128]], base=0, channel_multiplier=-1)
            ident_r = constp.tile([128, 128], F32R)
            nc.vector.tensor_scalar(
                out=ident_r[:], in0=io_t[:], scalar1=0, scalar2=None, op0=OP.is_equal
            )
            ident_f = constp.tile([128, 128], F32)
            nc.vector.tensor_scalar(
                out=ident_f[:], in0=io_t[:], scalar1=0, scalar2=None, op0=OP.is_equal
            )
            ones_col = constp.tile([128, 1], F32)
            nc.vector.memset(ones_col[:], 1.0)
            ones_row = constp.tile([1, 128], F32)
            nc.vector.memset(ones_row[:], 1.0)
            rows_sb = constp.tile([128, 32], I32)
            nc.sync.dma_start(rows_sb[:], rows_d[:])
            b1_sb = constp.tile([128, FC, C], F32)
            nc.sync.dma_start(b1_sb[:], b1_d[:])
            gam_sb = constp.tile([128, FC, C], F32)
            nc.sync.dma_start(gam_sb[:], gamma_d[:])
            bet_sb = constp.tile([128, FC, C], F32)
            nc.sync.dma_start(bet_sb[:], beta_d[:])
            loss_sb = constp.tile([128, C * IC], F32)
            bneps_sb = constp.tile([128, 1], F32)
            nc.vector.memset(bneps_sb[:], BN_EPS)
            epstau_sb = constp.tile([128, 1], F32)
            nc.vector.memset(epstau_sb[:], NORM_EPS * TAU)

            # ---------------- Pass 1: encoder -> mmn / mmnT / mm_out ----------
            with tc.tile_pool(name="p1_mmT", bufs=1) as pool_mmT, \
                 tc.tile_pool(name="p1_w", bufs=2) as pool_w, \
                 tc.tile_pool(name="p1_big", bufs=1) as pool_big, \
                 tc.tile_pool(name="p1_h1", bufs=2) as pool_h1, \
                 tc.tile_pool(name="p1_row", bufs=2) as pool_row, \
                 tc.tile_pool(name="p1_small", bufs=2) as pool_small, \
                 tc.tile_pool(name="ps_h1", bufs=2, space="PSUM") as ps_h1, \
                 tc.tile_pool(name="ps_h2", bufs=2, space="PSUM") as ps_h2, \
                 tc.tile_pool(name="ps_tr", bufs=2, space="PSUM") as ps_tr, \
                 tc.tile_pool(name="ps_bc", bufs=1, space="PSUM") as ps_bc:
                mmTs = pool_mmT.tile([128, KC, N], F32R)
                nc.sync.dma_start(mmTs[:], mmT_d.rearrange("(kc p) n -> p kc n", p=128))

                for c in range(C):
                    w1s = pool_w.tile([128, KC, R], F32R, tag="w1")
                    nc.sync.dma_start(
                        w1s[:], w1t_d[c].rearrange("(kc p) r -> p kc r", p=128)
                    )
                    w2s = pool_w.tile([128, FC, R], F32R, tag="w2")
                    nc.sync.dma_start(
                        w2s[:], w2t_d[c].rearrange("(ec p) r -> p ec r", p=128)
                    )
                    h2T = pool_big.tile([128, FC, N], F32R, tag="big")
                    stats = pool_small.tile([128, FC, NB, 6], F32, tag="stats")

                    for jb in range(NB):
                        jsl = slice(jb * 512, (jb + 1) * 512)
                        h1 = pool_h1.tile([128, FC, 512], F32R, tag="h1")
                        for ec in range(FC):
                            ps1 = ps_h1.tile([128, 512], F32, tag="ps1")
                            for kc in range(KC):
                                nc.tensor.matmul(
                                    ps1[:],
                                    w1s[:, kc, ec * 128:(ec + 1) * 128],
                                    mmTs[:, kc, jsl],
                                    start=(kc == 0),
                                    stop=(kc == KC - 1),
                                )
                            nc.scalar.activation(
                                h1[:, ec, :], ps1[:], AF.Gelu,
                                bias=b1_sb[:, ec, c:c + 1],
                            )
                        for fc in range(FC):
                            ps2 = ps_h2.tile([128, 512], F32, tag="ps2")
                            for ec in range(FC):
                                nc.tensor.matmul(
                                    ps2[:],
                                    w2s[:, ec, fc * 128:(fc + 1) * 128],
                                    h1[:, ec, :],
                                    start=(ec == 0),
                                    stop=(ec == FC - 1),
                                )
                            nc.vector.bn_stats(stats[:, fc, jb, :], ps2[:])
                            nc.scalar.copy(h2T[:, fc, jsl], ps2[:])

                    # BatchNorm parameters per feature
                    msv = pool_small.tile([128, FC, 2], F32, tag="msv")
                    s_sb = pool_small.tile([128, FC], F32, tag="s_sb")
                    bb_sb = pool_small.tile([128, FC], F32, tag="bb_sb")
                    for fc in range(FC):
                        nc.vector.bn_aggr(msv[:, fc, :], stats[:, fc, :, :])
                        std = pool_small.tile([128, 1], F32, tag="std")
                        nc.scalar.activation(std[:], msv[:, fc, 1:2], AF.Sqrt,
                                             bias=bneps_sb[:])
                        istd = pool_small.tile([128, 1], F32, tag="istd")
                        nc.vector.reciprocal(istd[:], std[:])
                        nc.vector.tensor_tensor(
                            out=s_sb[:, fc:fc + 1], in0=istd[:],
                            in1=gam_sb[:, fc, c:c + 1], op=OP.mult,
                        )
                        ms = pool_small.tile([128, 1], F32, tag="ms")
                        nc.vector.tensor_tensor(
                            out=ms[:], in0=msv[:, fc, 0:1],
                            in1=s_sb[:, fc:fc + 1], op=OP.mult,
                        )
                        nc.vector.tensor_tensor(
                            out=bb_sb[:, fc:fc + 1], in0=bet_sb[:, fc, c:c + 1],
                            in1=ms[:], op=OP.subtract,
                        )

                    # mm_c blocks -> transposed rows -> mm_out, norms, mmn, mmnT
                    for jb in range(NB):
                        jsl = slice(jb * 512, (jb + 1) * 512)
                        mmc = pool_h1.tile([128, FC, 512], F32, tag="mmc")
                        for fc in range(FC):
                            nc.scalar.activation(
                                mmc[:, fc, :], h2T[:, fc, jsl], AF.Identity,
                                bias=bb_sb[:, fc:fc + 1], scale=s_sb[:, fc:fc + 1],
                            )
                        mmblk = pool_row.tile([128, 4, R], F32, tag="mmblk")
                        for nb in range(4):
                            for fc in range(FC):
                                pst = ps_tr.tile([128, 128], F32, tag="pst")
                                nc.tensor.transpose(
                                    pst[:], mmc[:, fc, nb * 128:(nb + 1) * 128],
                                    ident_f[:],
                                )
                                nc.vector.tensor_copy(
                                    mmblk[:, nb, fc * 128:(fc + 1) * 128], pst[:]
                                )
                        for nb in range(4):
                            nc.gpsimd.indirect_dma_start(
                                out=mmout_d[c][:],
                                out_offset=bass.IndirectOffsetOnAxis(
                                    ap=rows_sb[:, jb * 4 + nb: jb * 4 + nb + 1],
                                    axis=0,
                                ),
                                in_=mmblk[:, nb, :],
                                in_offset=None,
                                bounds_check=N + 127,
                                oob_is_err=False,
                            )
                        # row norms for this block of 512 mm rows
                        sq = pool_row.tile([128, 4, R], F32, tag="sq")
                        nc.scalar.activation(sq[:], mmblk[:], AF.Square)
                        nsq = pool_small.tile([128, 4], F32, tag="nsq")
                        nc.vector.tensor_reduce(
                            nsq[:], sq[:], axis=mybir.AxisListType.X, op=OP.add
                        )
                        nrm = pool_small.tile([128, 4], F32, tag="nrm")
                        nc.scalar.activation(nrm[:], nsq[:], AF.Sqrt)
                        nc.vector.tensor_scalar_add(nrm[:], nrm[:], NORM_EPS)
                        rn = pool_small.tile([128, 4], F32, tag="rn")
                        nc.vector.reciprocal(rn[:], nrm[:])
                        # normalized rows -> mmn_dram
                        mmnb = pool_row.tile([128, 4, R], F32R, tag="mmnb")
                        for nb in range(4):
                            nc.vector.tensor_scalar_mul(
                                mmnb[:, nb, :], mmblk[:, nb, :], rn[:, nb:nb + 1]
                            )
                        nc.sync.dma_start(
                            mmn_dram[c][jsl, :].rearrange(
                                "(nb p) r -> p nb r", p=128
                            ),
                            mmnb[:],
                        )
                        # rn -> row layout [1, 512] via DRAM bounce
                        nc.sync.dma_start(rn_dram[:], rn[:])
                        rnrow = pool_small.tile([1, 512], F32, tag="rnrow")
                        nc.sync.dma_start(
                            rnrow[0:1, :].rearrange("o (t p) -> o t p", p=128),
                            rn_dram.rearrange("p t -> t p"),
                        )
                        # broadcast rn over partitions, scale columns -> mmnT
                        psb = ps_bc.tile([128, 512], F32, tag="psb")
                        nc.tensor.matmul(
                            psb[:], ones_row[:], rnrow[:], start=True, stop=True
                        )
                        for fc in range(FC):
                            nc.vector.tensor_tensor(
                                out=h2T[:, fc, jsl], in0=mmc[:, fc, :],
                                in1=psb[:], op=OP.mult,
                            )
                    nc.sync.dma_start(mmnT_dram[c], h2T[:])

            # ---------------- Pass 2: similarity / knn / loss ----------------
            with tc.tile_pool(name="p2_mnt", bufs=1) as pool_mnt, \
                 tc.tile_pool(name="p2_st", bufs=2) as pool_st, \
                 tc.tile_pool(name="p2_sme", bufs=2) as pool_sme, \
                 tc.tile_pool(name="p2_mrow", bufs=2) as pool_mrow, \
                 tc.tile_pool(name="p2_zap", bufs=2) as pool_zap, \
                 tc.tile_pool(name="p2_junk", bufs=2) as pool_junk, \
                 tc.tile_pool(name="p2_small", bufs=3) as p2s, \
                 tc.tile_pool(name="ps_sm", bufs=2, space="PSUM") as ps_sm, \
                 tc.tile_pool(name="ps_mr", bufs=2, space="PSUM") as ps_mr, \
                 tc.tile_pool(name="ps_t2", bufs=2, space="PSUM") as ps_t2:
                for c in range(C):
                    mns = pool_mnt.tile([128, FC, N], F32R, tag="mnt")
                    nc.sync.dma_start(mns[:], mmnT_dram[c])
                    sts = pool_st.tile([128, FC, S], F32R, tag="sts")
                    nc.sync.dma_start(
                        sts[:], stT_d[c].rearrange("(fc p) s -> p fc s", p=128)
                    )
                    # stru row norms -> exp scale rs_tau = 1/((||st||+eps)*tau)
                    sqs = pool_st.tile([128, FC, S], F32, tag="sqs")
                    nc.scalar.activation(sqs[:], sts[:], AF.Square)
                    rsn = p2s.tile([128, IC], F32, tag="rsn")
                    for ic in range(IC):
                        psn = ps_t2.tile([128, 1], F32, tag="psn")
                        for fc in range(FC):
                            nc.tensor.matmul(
                                psn[:],
                                sqs[:, fc, ic * 128:(ic + 1) * 128],
                                ones_col[:],
                                start=(fc == 0),
                                stop=(fc == FC - 1),
                            )
                        nc.vector.tensor_copy(rsn[:, ic:ic + 1], psn[:])
                    srt = p2s.tile([128, IC], F32, tag="srt")
                    nc.scalar.activation(srt[:], rsn[:], AF.Sqrt)
                    srt2 = p2s.tile([128, IC], F32, tag="srt2")
                    nc.scalar.activation(srt2[:], srt[:], AF.Identity,
                                         scale=TAU, bias=epstau_sb[:])
                    rst = p2s.tile([128, IC], F32, tag="rst")
                    nc.vector.reciprocal(rst[:], srt2[:])

                    for ic in range(IC):
                        isl = slice(ic * 128, (ic + 1) * 128)
                        sme = pool_sme.tile([128, N], F32, tag="sme")
                        rsp = p2s.tile([128, NB], F32, tag="rsp")
                        for jb in range(NB):
                            jsl = slice(jb * 512, (jb + 1) * 512)
                            ps = ps_sm.tile([128, 512], F32, tag="pssm")
                            for fc in range(FC):
                                nc.tensor.matmul(
                                    ps[:], sts[:, fc, isl], mns[:, fc, jsl],
                                    start=(fc == 0), stop=(fc == FC - 1),
                                )
                            nc.scalar.activation(
                                sme[:, jsl], ps[:], AF.Exp,
                                scale=rst[:, ic:ic + 1],
                                accum_out=rsp[:, jb:jb + 1],
                            )
                        rowsum = p2s.tile([128, 1], F32, tag="rowsum")
                        nc.vector.tensor_reduce(
                            rowsum[:], rsp[:], axis=mybir.AxisListType.X, op=OP.add
                        )
                        # argmax of the sim row
                        mx8 = p2s.tile([128, 8], F32, tag="mx8")
                        nc.vector.max(out=mx8[:], in_=sme[:])
                        mi8 = p2s.tile([128, 8], U32, tag="mi8")
                        nc.vector.max_index(out=mi8[:], in_max=mx8[:], in_values=sme[:])
                        idx32 = p2s.tile([128, 1], I32, tag="idx32")
                        nc.vector.tensor_copy(idx32[:], mi8[:, 0:1])
                        # gather mmn[argmax] and transpose
                        gat = p2s.tile([128, R], F32R, tag="gat")
                        nc.gpsimd.indirect_dma_start(
                            out=gat[:], out_offset=None, in_=mmn_dram[c][:],
                            in_offset=bass.IndirectOffsetOnAxis(
                                ap=idx32[:, :1], axis=0
                            ),
                        )
                        gmT = p2s.tile([128, FC, 128], F32R, tag="gmT")
                        for fc in range(FC):
                            pst2 = ps_t2.tile([128, 128], F32R, tag="pst2")
                            nc.tensor.transpose(
                                pst2[:], gat[:, fc * 128:(fc + 1) * 128], ident_r[:]
                            )
                            nc.vector.tensor_copy(gmT[:, fc, :], pst2[:])
                        # mm-rows block
                        mrow = pool_mrow.tile([128, N], F32, tag="mrow")
                        for jb in range(NB):
                            jsl = slice(jb * 512, (jb + 1) * 512)
                            ps = ps_mr.tile([128, 512], F32, tag="psmr")
                            for fc in range(FC):
                                nc.tensor.matmul(
                                    ps[:], gmT[:, fc, :], mns[:, fc, jsl],
                                    start=(fc == 0), stop=(fc == FC - 1),
                                )
                            nc.scalar.copy(mrow[:, jsl], ps[:])
                        # top-6 mask and pos sum
                        m8b = p2s.tile([128, 8], F32, tag="m8b")
                        nc.vector.max(out=m8b[:], in_=mrow[:])
                        m6 = p2s.tile([128, 8], F32, tag="m6")
                        nc.vector.tensor_copy(m6[:], m8b[:])
                        nc.vector.memset(m6[:, 6:8], BIGNEG)
                        zap = pool_zap.tile([128, N], F32, tag="zap")
                        nc.vector.match_replace(
                            out=zap[:], in_to_replace=m6[:], in_values=mrow[:],
                            imm_value=BIGNEG,
                        )
                        nc.vector.tensor_tensor(
                            out=zap[:], in0=mrow[:], in1=zap[:], op=OP.is_gt
                        )
                        nc.vector.tensor_tensor(
                            out=zap[:], in0=zap[:], in1=sme[:], op=OP.mult
                        )
                        junk = pool_junk.tile([128, N], F32, tag="junk")
                        pos = p2s.tile([128, 1], F32, tag="pos")
                        nc.scalar.activation(junk[:], zap[:], AF.Identity,
                                             accum_out=pos[:])
                        # loss terms: log(rowsum) - log(pos)
                        lr = p2s.tile([128, 1], F32, tag="lr")
                        nc.scalar.activation(lr[:], rowsum[:], AF.Ln)
                        lp = p2s.tile([128, 1], F32, tag="lp")
                        nc.scalar.activation(lp[:], pos[:], AF.Ln)
                        nc.vector.tensor_tensor(
                            out=loss_sb[:, c * IC + ic: c * IC + ic + 1],
                            in0=lr[:], in1=lp[:], op=OP.subtract,
                        )
            nc.sync.dma_start(loss_d[:], loss_sb[:])

    nc.compile()
    return nc


_NC = None


def _get_nc():
    global _NC
    if _NC is None:
        _NC = build_program()
    return _NC


def _prepare_in_maps(st_feats, mm_feats, W1, b1, W2, b2, gamma, beta):
    st = np.asarray(st_feats, dtype=np.float32)
    mm = np.asarray(mm_feats, dtype=np.float32)
    W1 = np.asarray(W1, dtype=np.float32)
    W2 = np.asarray(W2, dtype=np.float32)
    def _prm(x):
        # [C, R] -> [128, FC, C] with element [p, fc, c] = x[c, fc*128 + p]
        x = np.asarray(x, dtype=np.float32).reshape(C, FC, 128)
        return np.ascontiguousarray(x.transpose(2, 1, 0))

    b1 = _prm(b1)
    gamma = _prm(gamma)
    beta = _prm(beta)

    mmT = np.ascontiguousarray(mm.T)                    # [768, 4096]
    w1t = np.ascontiguousarray(W1.transpose(0, 2, 1))   # [5, 768, 256]
    w2t = np.ascontiguousarray(W2.transpose(0, 2, 1))   # [5, 256, 256]

    in_maps = []
    for k in range(NCORES):
        stT = np.ascontiguousarray(
            st[k * S:(k + 1) * S].transpose(2, 1, 0)    # [5, 256, 512]
        )
        rows = np.empty((128, 32), dtype=np.int32)
        for jb in range(NB):
            for nb in range(4):
                j0 = jb * 512 + nb * 128
                if jb == k:
                    rows[:, jb * 4 + nb] = j0 + np.arange(128)
                else:
                    rows[:, jb * 4 + nb] = N + np.arange(128)
        in_maps.append({
            "mmT": mmT, "w1t": w1t, "w2t": w2t, "b1": b1,
            "gamma": gamma, "beta": beta, "stT": stT, "rows": rows,
        })
    return in_maps


def kernel(st_feats, mm_feats, W1, b1, W2, b2, gamma, beta):
    nc = _get_nc()
    in_maps = _prepare_in_maps(st_feats, mm_feats, W1, b1, W2, b2, gamma, beta)
    res = run_bass_kernel_spmd(nc, in_maps, core_ids=list(range(NCORES)))

    mm_out = np.empty((N, R, C), dtype=np.float32)
    loss_total = 0.0
    for k in range(NCORES):
        rk = res.results[k]
        for c in range(C):
            mm_out[k * S:(k + 1) * S, :, c] = rk[f"mm_out_{c}"][k * S:(k + 1) * S]
        loss_total += rk["loss"].astype(np.float64).sum()
    loss = np.float32(loss_total / (C * N))
    return mm_out, loss


# revision 11
# speedup vs baseline: 1.7066x; 1.7066x over previous
"""Trainium2 Bass kernel for nn_LoretzFusion (retrieval_knn).

Strategy (8 NeuronCores, SPMD via run_bass_kernel_spmd):
  - Encoder (5 parallel MLPs + BatchNorm over N) is replicated on every
    core (cheap relative to the N x N similarity work, avoids collectives).
  - The N x N similarity work is sharded by stru rows: core k owns rows
    [k*512, (k+1)*512). Each core computes its [512, 4096] exp-sim block,
    row-sums, argmax, gathers mmn[argmax] (indirect DMA), computes its
    [512, 4096] mm-rows block, top-6 via max8/match_replace mask, and the
    per-row loss terms.
  - Host slices/transposes inputs, and merges per-core outputs (concat of
    row slices + scalar mean for the loss).
All matmuls use float32r (full-rate fp32 mode, ~1e-3 rel precision).
"""

import sys
import os

sys.path.insert(0, "/opt/trn_rl_repo")

import numpy as np
import concourse.bacc as bacc
import concourse.bass as bass
import concourse.tile as tile
from concourse import mybir
from concourse.bass_utils import run_bass_kernel_spmd

F32 = mybir.dt.float32
F32R = mybir.dt.float32r
I32 = mybir.dt.int32
U32 = mybir.dt.uint32
AF = mybir.ActivationFunctionType
OP = mybir.AluOpType

NCORES = 8
N, R, MM, C = 4096, 256, 768, 5
S = N // NCORES          # 512 stru rows per core
NB = N // 512            # 8 column blocks of 512
FC = R // 128            # 2 chunks of the 256-dim feature axis
KC = MM // 128           # 6 chunks of the 768-dim contraction
IC = S // 128            # 4 chunks of this core's 512 rows
TAU = 0.5
BN_EPS = 1e-5
NORM_EPS = 1e-12
BIGNEG = -3.0e38
OOB = 1 << 30


def build_program():
    nc = bacc.Bacc("TRN2", target_bir_lowering=False, debug=True)

    mmT_d = nc.dram_tensor("mmT", [MM, N], F32R, kind="ExternalInput")
    w1t_d = nc.dram_tensor("w1t", [C, MM, R], F32R, kind="ExternalInput")
    w2t_d = nc.dram_tensor("w2t", [C, R, R], F32R, kind="ExternalInput")
    b1_d = nc.dram_tensor("b1", [128, FC, C], F32, kind="ExternalInput")
    gamma_d = nc.dram_tensor("gamma", [128, FC, C], F32, kind="ExternalInput")
    beta_d = nc.dram_tensor("beta", [128, FC, C], F32, kind="ExternalInput")
    stT_d = nc.dram_tensor("stT", [C, R, S], F32R, kind="ExternalInput")
    rows_d = nc.dram_tensor("rows", [128, 32], I32, kind="ExternalInput")

    # S + 128 guard rows: foreign blocks scatter into the guard region
    mmout_d = [
        nc.dram_tensor(f"mm_out_{c}", [S + 128, R], F32, kind="ExternalOutput")
        for c in range(C)
    ]
    loss_d = nc.dram_tensor("loss", [128, C * IC], F32, kind="ExternalOutput")

    mmnT_dram = nc.dram_tensor("mmnT_i", [C, 128, FC, N], F32R)
    mmn_dram = [nc.dram_tensor(f"mmn_i_{c}", [N, R], F32R) for c in range(C)]
    rn_dram = nc.dram_tensor("rn_i", [128, 4], F32)

    with tile.TileContext(nc) as tc:
        with tc.tile_pool(name="const", bufs=1) as constp:
            # identity matrices for PE transposes
            io_t = constp.tile([128, 128], I32)
            nc.gpsimd.iota(io_t[:], pattern=[[1, 128]], base=0, channel_multiplier=-1)
            ident_r = constp.tile([128, 128], F32R)
            nc.vector.tensor_scalar(
                out=ident_r[:], in0=io_t[:], scalar1=0, scalar2=None, op0=OP.is_equal
            )
            ident_f = constp.tile([128, 128], F32)
            nc.vector.tensor_scalar(
                out=ident_f[:], in0=io_t[:], scalar1=0, scalar2=None, op0=OP.is_equal
            )
            ones_col = constp.tile([128, 1], F32)
            nc.vector.memset(ones_col[:], 1.0)
            ones_row = constp.tile([1, 128], F32)
            nc.vector.memset(ones_row[:], 1.0)
            rows_sb = constp.tile([128, 32], I32)
            nc.sync.dma_start(rows_sb[:], rows_d[:])
            b1_sb = constp.tile([128, FC, C], F32)
            nc.sync.dma_start(b1_sb[:], b1_d[:])
            gam_sb = constp.tile([128, FC, C], F32)
            nc.sync.dma_start(gam_sb[:], gamma_d[:])
            bet_sb = constp.tile([128, FC, C], F32)
            nc.sync.dma_start(bet_sb[:], beta_d[:])
            loss_sb = constp.tile([128, C * IC], F32)
            bneps_sb = constp.tile([128, 1], F32)
            nc.vector.memset(bneps_sb[:], BN_EPS)
            epstau_sb = constp.tile([128, 1], F32)
            nc.vector.memset(epstau_sb[:], NORM_EPS * TAU)

            # ---------------- Pass 1: encoder -> mmn / mmnT / mm_out ----------
            with tc.tile_pool(name="p1_mmT", bufs=1) as pool_mmT, \
                 tc.tile_pool(name="p1_w", bufs=2) as pool_w, \
                 tc.tile_pool(name="p1_big", bufs=1) as pool_big, \
                 tc.tile_pool(name="p1_h1", bufs=2) as pool_h1, \
                 tc.tile_pool(name="p1_row", bufs=2) as pool_row, \
                 tc.tile_pool(name="p1_small", bufs=2) as pool_small, \
                 tc.tile_pool(name="ps_h1", bufs=2, space="PSUM") as ps_h1, \
                 tc.tile_pool(name="ps_h2", bufs=2, space="PSUM") as ps_h2, \
                 tc.tile_pool(name="ps_tr", bufs=2, space="PSUM") as ps_tr, \
                 tc.tile_pool(name="ps_bc", bufs=1, space="PSUM") as ps_bc:
                mmTs = pool_mmT.tile([128, KC, N], F32R)
                nc.sync.dma_start(mmTs[:], mmT_d.rearrange("(kc p) n -> p kc n", p=128))

                for c in range(C):
                    w1s = pool_w.tile([128, KC, R], F32R, tag="w1")
                    nc.sync.dma_start(
                        w1s[:], w1t_d[c].rearrange("(kc p) r -> p kc r", p=128)
                    )
                    w2s = pool_w.tile([128, FC, R], F32R, tag="w2")
                    nc.sync.dma_start(
                        w2s[:], w2t_d[c].rearrange("(ec p) r -> p ec r", p=128)
                    )
                    h2T = pool_big.tile([128, FC, N], F32R, tag="big")
                    stats = pool_small.tile([128, FC, NB, 6], F32, tag="stats")

                    for jb in range(NB):
                        jsl = slice(jb * 512, (jb + 1) * 512)
                        h1 = pool_h1.tile([128, FC, 512], F32R, tag="h1")
                        for ec in range(FC):
                            ps1 = ps_h1.tile([128, 512], F32, tag="ps1")
                            for kc in range(KC):
                                nc.tensor.matmul(
                                    ps1[:],
                                    w1s[:, kc, ec * 128:(ec + 1) * 128],
                                    mmTs[:, kc, jsl],
                                    start=(kc == 0),
                                    stop=(kc == KC - 1),
                                )
                            nc.scalar.activation(
                                h1[:, ec, :], ps1[:], AF.Gelu,
                                bias=b1_sb[:, ec, c:c + 1],
                            )
                        for fc in range(FC):
                            ps2 = ps_h2.tile([128, 512], F32, tag="ps2")
                            for ec in range(FC):
                                nc.tensor.matmul(
                                    ps2[:],
                                    w2s[:, ec, fc * 128:(fc + 1) * 128],
                                    h1[:, ec, :],
                                    start=(ec == 0),
                                    stop=(ec == FC - 1),
                                )
                            nc.vector.bn_stats(stats[:, fc, jb, :], ps2[:])
                            nc.scalar.copy(h2T[:, fc, jsl], ps2[:])

                    # BatchNorm parameters per feature
                    msv = pool_small.tile([128, FC, 2], F32, tag="msv")
                    s_sb = pool_small.tile([128, FC], F32, tag="s_sb")
                    bb_sb = pool_small.tile([128, FC], F32, tag="bb_sb")
                    for fc in range(FC):
                        nc.vector.bn_aggr(msv[:, fc, :], stats[:, fc, :, :])
                        std = pool_small.tile([128, 1], F32, tag="std")
                        nc.scalar.activation(std[:], msv[:, fc, 1:2], AF.Sqrt,
                                             bias=bneps_sb[:])
                        istd = pool_small.tile([128, 1], F32, tag="istd")
                        nc.vector.reciprocal(istd[:], std[:])
                        nc.vector.tensor_tensor(
                            out=s_sb[:, fc:fc + 1], in0=istd[:],
                            in1=gam_sb[:, fc, c:c + 1], op=OP.mult,
                        )
                        ms = pool_small.tile([128, 1], F32, tag="ms")
                        nc.vector.tensor_tensor(
                            out=ms[:], in0=msv[:, fc, 0:1],
                            in1=s_sb[:, fc:fc + 1], op=OP.mult,
                        )
                        nc.vector.tensor_tensor(
                            out=bb_sb[:, fc:fc + 1], in0=bet_sb[:, fc, c:c + 1],
                            in1=ms[:], op=OP.subtract,
                        )

                    # mm_c blocks -> transposed rows -> mm_out, norms, mmn, mmnT
                    for jb in range(NB):
                        jsl = slice(jb * 512, (jb + 1) * 512)
                        mmc = pool_h1.tile([128, FC, 512], F32, tag="mmc")
                        for fc in range(FC):
                            nc.scalar.activation(
                                mmc[:, fc, :], h2T[:, fc, jsl], AF.Identity,
                                bias=bb_sb[:, fc:fc + 1], scale=s_sb[:, fc:fc + 1],
                            )
                        mmblk = pool_row.tile([128, 4, R], F32, tag="mmblk")
                        for nb in range(4):
                            for fc in range(FC):
                                pst = ps_tr.tile([128, 128], F32, tag="pst")
                                nc.tensor.transpose(
                                    pst[:], mmc[:, fc, nb * 128:(nb + 1) * 128],
                                    ident_f[:],
                                )
                                nc.vector.tensor_copy(
                                    mmblk[:, nb, fc * 128:(fc + 1) * 128], pst[:]
                                )
                        for nb in range(4):
                            nc.gpsimd.indirect_dma_start(
                                out=mmout_d[c][:],
                                out_offset=bass.IndirectOffsetOnAxis(
                                    ap=rows_sb[:, jb * 4 + nb: jb * 4 + nb + 1],
                                    axis=0,
                                ),
                                in_=mmblk[:, nb, :],
                                in_offset=None,
                                bounds_check=S + 127,
                                oob_is_err=False,
                            )
                        # row norms for this block of 512 mm rows
                        sq = pool_row.tile([128, 4, R], F32, tag="sq")
                        nc.scalar.activation(sq[:], mmblk[:], AF.Square)
                        nsq = pool_small.tile([128, 4], F32, tag="nsq")
                        nc.vector.tensor_reduce(
                            nsq[:], sq[:], axis=mybir.AxisListType.X, op=OP.add
                        )
                        nrm = pool_small.tile([128, 4], F32, tag="nrm")
                        nc.scalar.activation(nrm[:], nsq[:], AF.Sqrt)
                        nc.vector.tensor_scalar_add(nrm[:], nrm[:], NORM_EPS)
                        rn = pool_small.tile([128, 4], F32, tag="rn")
                        nc.vector.reciprocal(rn[:], nrm[:])
                        # normalized rows -> mmn_dram
                        mmnb = pool_row.tile([128, 4, R], F32R, tag="mmnb")
                        for nb in range(4):
                            nc.vector.tensor_scalar_mul(
                                mmnb[:, nb, :], mmblk[:, nb, :], rn[:, nb:nb + 1]
                            )
                        nc.sync.dma_start(
                            mmn_dram[c][jsl, :].rearrange(
                                "(nb p) r -> p nb r", p=128
                            ),
                            mmnb[:],
                        )
                        # rn -> row layout [1, 512] via DRAM bounce
                        nc.sync.dma_start(rn_dram[:], rn[:])
                        rnrow = pool_small.tile([1, 512], F32, tag="rnrow")
                        nc.sync.dma_start(
                            rnrow[0:1, :].rearrange("o (t p) -> o t p", p=128),
                            rn_dram.rearrange("p t -> t p"),
                        )
                        # broadcast rn over partitions, scale columns -> mmnT
                        psb = ps_bc.tile([128, 512], F32, tag="psb")
                        nc.tensor.matmul(
                            psb[:], ones_row[:], rnrow[:], start=True, stop=True
                        )
                        for fc in range(FC):
                            nc.vector.tensor_tensor(
                                out=h2T[:, fc, jsl], in0=mmc[:, fc, :],
                                in1=psb[:], op=OP.mult,
                            )
                    nc.sync.dma_start(mmnT_dram[c], h2T[:])

            # ---------------- Pass 2: similarity / knn / loss ----------------
            with tc.tile_pool(name="p2_mnt", bufs=1) as pool_mnt, \
                 tc.tile_pool(name="p2_st", bufs=2) as pool_st, \
                 tc.tile_pool(name="p2_sme", bufs=2) as pool_sme, \
                 tc.tile_pool(name="p2_mrow", bufs=2) as pool_mrow, \
                 tc.tile_pool(name="p2_zap", bufs=2) as pool_zap, \
                 tc.tile_pool(name="p2_junk", bufs=2) as pool_junk, \
                 tc.tile_pool(name="p2_small", bufs=3) as p2s, \
                 tc.tile_pool(name="ps_sm", bufs=2, space="PSUM") as ps_sm, \
                 tc.tile_pool(name="ps_mr", bufs=2, space="PSUM") as ps_mr, \
                 tc.tile_pool(name="ps_t2", bufs=2, space="PSUM") as ps_t2:
                for c in range(C):
                    mns = pool_mnt.tile([128, FC, N], F32R, tag="mnt")
                    nc.sync.dma_start(mns[:], mmnT_dram[c])
                    sts = pool_st.tile([128, FC, S], F32R, tag="sts")
                    nc.sync.dma_start(
                        sts[:], stT_d[c].rearrange("(fc p) s -> p fc s", p=128)
                    )
                    # stru row norms -> exp scale rs_tau = 1/((||st||+eps)*tau)
                    sqs = pool_st.tile([128, FC, S], F32, tag="sqs")
                    nc.scalar.activation(sqs[:], sts[:], AF.Square)
                    rsn = p2s.tile([128, IC], F32, tag="rsn")
                    for ic in range(IC):
                        psn = ps_t2.tile([128, 1], F32, tag="psn")
                        for fc in range(FC):
                            nc.tensor.matmul(
                                psn[:],
                                sqs[:, fc, ic * 128:(ic + 1) * 128],
                                ones_col[:],
                                start=(fc == 0),
                                stop=(fc == FC - 1),
                            )
                        nc.vector.tensor_copy(rsn[:, ic:ic + 1], psn[:])
                    srt = p2s.tile([128, IC], F32, tag="srt")
                    nc.scalar.activation(srt[:], rsn[:], AF.Sqrt)
                    srt2 = p2s.tile([128, IC], F32, tag="srt2")
                    nc.scalar.activation(srt2[:], srt[:], AF.Identity,
                                         scale=TAU, bias=epstau_sb[:])
                    rst = p2s.tile([128, IC], F32, tag="rst")
                    nc.vector.reciprocal(rst[:], srt2[:])

                    for ic in range(IC):
                        isl = slice(ic * 128, (ic + 1) * 128)
                        sme = pool_sme.tile([128, N], F32, tag="sme")
                        rsp = p2s.tile([128, NB], F32, tag="rsp")
                        for jb in range(NB):
                            jsl = slice(jb * 512, (jb + 1) * 512)
                            ps = ps_sm.tile([128, 512], F32, tag="pssm")
                            for fc in range(FC):
                                nc.tensor.matmul(
                                    ps[:], sts[:, fc, isl], mns[:, fc, jsl],
                                    start=(fc == 0), stop=(fc == FC - 1),
                                )
                            nc.scalar.activation(
                                sme[:, jsl], ps[:], AF.Exp,
                                scale=rst[:, ic:ic + 1],
                                accum_out=rsp[:, jb:jb + 1],
                            )
                        rowsum = p2s.tile([128, 1], F32, tag="rowsum")
                        nc.vector.tensor_reduce(
                            rowsum[:], rsp[:], axis=mybir.AxisListType.X, op=OP.add
                        )
                        # argmax of the sim row
                        mx8 = p2s.tile([128, 8], F32, tag="mx8")
                        nc.vector.max(out=mx8[:], in_=sme[:])
                        mi8 = p2s.tile([128, 8], U32, tag="mi8")
                        nc.vector.max_index(out=mi8[:], in_max=mx8[:], in_values=sme[:])
                        idx32 = p2s.tile([128, 1], I32, tag="idx32")
                        nc.vector.tensor_copy(idx32[:], mi8[:, 0:1])
                        # gather mmn[argmax] and transpose
                        gat = p2s.tile([128, R], F32R, tag="gat")
                        nc.gpsimd.indirect_dma_start(
                            out=gat[:], out_offset=None, in_=mmn_dram[c][:],
                            in_offset=bass.IndirectOffsetOnAxis(
                                ap=idx32[:, :1], axis=0
                            ),
                        )
                        gmT = p2s.tile([128, FC, 128], F32R, tag="gmT")
                        for fc in range(FC):
                            pst2 = ps_t2.tile([128, 128], F32R, tag="pst2")
                            nc.tensor.transpose(
                                pst2[:], gat[:, fc * 128:(fc + 1) * 128], ident_r[:]
                            )
                            nc.vector.tensor_copy(gmT[:, fc, :], pst2[:])
                        # mm-rows block
                        mrow = pool_mrow.tile([128, N], F32, tag="mrow")
                        for jb in range(NB):
                            jsl = slice(jb * 512, (jb + 1) * 512)
                            ps = ps_mr.tile([128, 512], F32, tag="psmr")
                            for fc in range(FC):
                                nc.tensor.matmul(
                                    ps[:], gmT[:, fc, :], mns[:, fc, jsl],
                                    start=(fc == 0), stop=(fc == FC - 1),
                                )
                            nc.scalar.copy(mrow[:, jsl], ps[:])
                        # top-6 mask and pos sum
                        m8b = p2s.tile([128, 8], F32, tag="m8b")
                        nc.vector.max(out=m8b[:], in_=mrow[:])
                        m6 = p2s.tile([128, 8], F32, tag="m6")
                        nc.vector.tensor_copy(m6[:], m8b[:])
                        nc.vector.memset(m6[:, 6:8], BIGNEG)
                        zap = pool_zap.tile([128, N], F32, tag="zap")
                        nc.vector.match_replace(
                            out=zap[:], in_to_replace=m6[:], in_values=mrow[:],
                            imm_value=BIGNEG,
                        )
                        nc.vector.tensor_tensor(
                            out=zap[:], in0=mrow[:], in1=zap[:], op=OP.is_gt
                        )
                        nc.vector.tensor_tensor(
                            out=zap[:], in0=zap[:], in1=sme[:], op=OP.mult
                        )
                        junk = pool_junk.tile([128, N], F32, tag="junk")
                        pos = p2s.tile([128, 1], F32, tag="pos")
                        nc.scalar.activation(junk[:], zap[:], AF.Identity,
                                             accum_out=pos[:])
                        # loss terms: log(rowsum) - log(pos)
                        lr = p2s.tile([128, 1], F32, tag="lr")
                        nc.scalar.activation(lr[:], rowsum[:], AF.Ln)
                        lp = p2s.tile([128, 1], F32, tag="lp")
                        nc.scalar.activation(lp[:], pos[:], AF.Ln)
                        nc.vector.tensor_tensor(
                            out=loss_sb[:, c * IC + ic: c * IC + ic + 1],
                            in0=lr[:], in1=lp[:], op=OP.subtract,
                        )
            nc.sync.dma_start(loss_d[:], loss_sb[:])

    nc.compile()
    return nc


_RUNNER = None


class _Runner:
    """Builds the SPMD executable once (same PJRT path as
    run_bass_kernel_spmd under axon) and reuses it across calls."""

    def __init__(self):
        import jax
        from jax.sharding import Mesh, PartitionSpec
        from jax.experimental.shard_map import shard_map
        from concourse import bass2jax, mybir as mb

        self.jax = jax
        nc = build_program()
        self.nc = nc
        bass2jax.install_neuronx_cc_hook()

        in_names, out_names, out_avals, zero_outs = [], [], [], []
        partition_name = (
            nc.partition_id_tensor.name if nc.partition_id_tensor else None
        )
        for alloc in nc.m.functions[0].allocations:
            if not isinstance(alloc, mb.MemoryLocationSet):
                continue
            name = alloc.memorylocations[0].name
            if alloc.kind == "ExternalInput":
                if name != partition_name:
                    in_names.append(name)
            elif alloc.kind == "ExternalOutput":
                out_names.append(name)
                shape = tuple(alloc.tensor_shape)
                dtype = mb.dt.np(alloc.dtype)
                out_avals.append(jax.core.ShapedArray(shape, dtype))
                zero_outs.append(np.zeros(shape, dtype))
        self.dbg_name = None
        if nc.dbg_addr is not None:
            self.dbg_name = nc.dbg_addr.name
        n_params = len(in_names)
        in_names = in_names + out_names
        if partition_name is not None:
            in_names.append(partition_name)

        def _body(*args):
            operands = list(args)
            if partition_name is not None:
                operands.append(bass2jax.partition_id_tensor())
            outs = bass2jax._bass_exec_p.bind(
                *operands,
                out_avals=tuple(out_avals),
                in_names=tuple(in_names),
                out_names=tuple(out_names),
                lowering_input_output_aliases=(),
                sim_require_finite=True,
                sim_require_nnan=True,
                nc=nc,
            )
            return tuple(outs)

        devices = jax.devices()[:NCORES]
        mesh = Mesh(np.asarray(devices), ("core",))
        n_io = n_params + len(out_names)
        donate = tuple(range(n_params, n_io))
        self.sharded = jax.jit(
            shard_map(
                _body, mesh=mesh,
                in_specs=(PartitionSpec("core"),) * n_io,
                out_specs=(PartitionSpec("core"),) * len(out_names),
                check_rep=False,
            ),
            donate_argnums=donate, keep_unused=True,
        )
        self.in_names = in_names[:n_params]
        self.out_names = out_names
        self.out_avals = out_avals
        self.zero_outs = zero_outs
        self._staged = None

    def stage_inputs(self, in_maps):
        if self.dbg_name is not None:
            in_maps = [
                {**m, self.dbg_name: np.zeros((1, 2), np.uint32)} for m in in_maps
            ]
        concat = [
            np.concatenate([np.asarray(m[name]) for m in in_maps], axis=0)
            for name in self.in_names
        ]
        self._staged = [self.jax.device_put(a) for a in concat]

    def execute(self):
        zeros = [
            np.zeros((NCORES * z.shape[0], *z.shape[1:]), z.dtype)
            for z in self.zero_outs
        ]
        out = self.sharded(*self._staged, *zeros)
        return out

    def run(self, in_maps):
        self.stage_inputs(in_maps)
        out_arrs = [np.asarray(o) for o in self.execute()]
        return [
            {
                name: out_arrs[i].reshape(NCORES, *self.out_avals[i].shape)[k]
                for i, name in enumerate(self.out_names)
            }
            for k in range(NCORES)
        ]


def _get_runner():
    global _RUNNER
    if _RUNNER is None:
        _RUNNER = _Runner()
    return _RUNNER


def _prepare_in_maps(st_feats, mm_feats, W1, b1, W2, b2, gamma, beta):
    st = np.asarray(st_feats, dtype=np.float32)
    mm = np.asarray(mm_feats, dtype=np.float32)
    W1 = np.asarray(W1, dtype=np.float32)
    W2 = np.asarray(W2, dtype=np.float32)
    def _prm(x):
        # [C, R] -> [128, FC, C] with element [p, fc, c] = x[c, fc*128 + p]
        x = np.asarray(x, dtype=np.float32).reshape(C, FC, 128)
        return np.ascontiguousarray(x.transpose(2, 1, 0))

    b1 = _prm(b1)
    gamma = _prm(gamma)
    beta = _prm(beta)

    mmT = np.ascontiguousarray(mm.T)                    # [768, 4096]
    w1t = np.ascontiguousarray(W1.transpose(0, 2, 1))   # [5, 768, 256]
    w2t = np.ascontiguousarray(W2.transpose(0, 2, 1))   # [5, 256, 256]

    in_maps = []
    for k in range(NCORES):
        stT = np.ascontiguousarray(
            st[k * S:(k + 1) * S].transpose(2, 1, 0)    # [5, 256, 512]
        )
        rows = np.empty((128, 32), dtype=np.int32)
        for jb in range(NB):
            for nb in range(4):
                if jb == k:
                    rows[:, jb * 4 + nb] = nb * 128 + np.arange(128)
                else:
                    rows[:, jb * 4 + nb] = S + np.arange(128)
        in_maps.append({
            "mmT": mmT, "w1t": w1t, "w2t": w2t, "b1": b1,
            "gamma": gamma, "beta": beta, "stT": stT, "rows": rows,
        })
    return in_maps


def kernel(st_feats, mm_feats, W1, b1, W2, b2, gamma, beta):
    runner = _get_runner()
    in_maps = _prepare_in_maps(st_feats, mm_feats, W1, b1, W2, b2, gamma, beta)
    results = runner.run(in_maps)

    mm_out = np.empty((N, R, C), dtype=np.float32)
    loss_total = 0.0
    for k in range(NCORES):
        rk = results[k]
        for c in range(C):
            mm_out[k * S:(k + 1) * S, :, c] = rk[f"mm_out_{c}"][:S]
        loss_total += rk["loss"].astype(np.float64).sum()
    loss = np.float32(loss_total / (C * N))
    return mm_out, loss
